# revision 11
# baseline (speedup 1.0000x reference)
"""Trainium2 Bass kernel for nn_MultiHeadAttention (B=4, S=2048, D=768, H=12).

Sharding: 8 cores = 4 batches x 2 head-groups (6 heads each).
Per core (batch b, group g), all bf16:
  KT/QT = Wg @ x_b^T            [384, 2048]  (bias added on DVE)
  V     = x_b @ Wv_g^T          16 tiles [128, 6*65] (65th col = 1.0 -> denom)
  attention per (qcp in 2, head-pair hp in 3):
    scores burst: 16 chunks; per chunk two row-tiled co-issued K=64 matmuls
      (head A rows 0-63, head B rows 64-127) -> sAB [128, 2048] PSUM
    exp: one ACT instr [128, 2048] per chunk -> E bf16 SBUF
    PV burst: per chunk 2 full-mode M=65 matmuls (V_aug^T @ E half),
      accumulating att[65, 1024] per head; row 64 = softmax denominator
    normalize: recip_approx_fast(denom row) -> gpsimd partition_broadcast
      -> DVE mult -> attn bf16
  o-proj per qcp: outT[768, 1024] bf16 (bias bo_eff = Wo_g @ bv_g + bo/2
  folded host-side; V projection carries no bias)
Host sums the two bf16 partial outT per batch (f32) and transposes back.
"""

import sys

import numpy as np
import ml_dtypes

if "/opt/trn_rl_repo" not in sys.path:
    sys.path.insert(0, "/opt/trn_rl_repo")

import concourse.bass as bass
import concourse.bacc as bacc
import concourse.mybir as mybir
import concourse.tile as tile
from concourse.bass_utils import run_bass_kernel_spmd

B, S, DM, NH, DK = 4, 2048, 768, 12, 64
NCORES = 8
HLOC = 6            # heads per core
GD = HLOC * DK      # 384
P = 128
NXT = DM // P       # 6 contraction tiles over d_model
NPT = GD // P       # 3 head-pair tiles (2 heads each)
NKT = S // P        # 16 k chunks
QC = 512            # q chunk
NQC = S // QC       # 4
VW = 65             # V cols per head (64 + ones)
VROW = HLOC * VW    # 390

F32 = mybir.dt.float32
BF16 = mybir.dt.bfloat16
EXP = mybir.ActivationFunctionType.Exp
NPBF16 = ml_dtypes.bfloat16

_NC_CACHE = {}


def build_nc():
    nc = bacc.Bacc()

    xT = nc.declare_dram_parameter("xT", [P, NXT * S], BF16, isOutput=False)
    wqT = nc.declare_dram_parameter("wqT", [P, NXT * GD], BF16, isOutput=False)
    wkT = nc.declare_dram_parameter("wkT", [P, NXT * GD], BF16, isOutput=False)
    wvT = nc.declare_dram_parameter("wvT", [P, NXT * GD], BF16, isOutput=False)
    woT = nc.declare_dram_parameter("woT", [P, NPT * DM], BF16, isOutput=False)
    pb = nc.declare_dram_parameter("pb", [P, 12], F32, isOutput=False)
    outT = nc.declare_dram_parameter("outT", [DM, S], BF16, isOutput=True)

    with tile.TileContext(nc) as tc:
        with (
            nc.allow_low_precision(reason="bf16 pipeline is intended"),
            tc.tile_pool(name="persist", bufs=1) as pp,
            tc.tile_pool(name="xpool", bufs=1) as xp,
            tc.tile_pool(name="epool", bufs=1) as ep,
            tc.tile_pool(name="work", bufs=1) as wp,
            tc.tile_pool(name="psum", bufs=1, space=bass.MemorySpace.PSUM) as psp,
        ):
            # ---- DMA loads (one per tensor; sync dispatch is serialized) ----
            wk_all = pp.tile([P, NXT, GD], BF16, tag="wk", name="wk_all")
            nc.sync.dma_start(wk_all[:], wkT[:])
            xt_all = xp.tile([P, NXT, S], BF16, tag="xt", name="xt_all")
            for i in range(NXT):
                nc.sync.dma_start(xt_all[:, i, :], xT[:, i * S : (i + 1) * S])
            wq_all = pp.tile([P, NXT, GD], BF16, tag="wq", name="wq_all")
            nc.sync.dma_start(wq_all[:], wqT[:])
            wv_all = pp.tile([P, NXT, GD], BF16, tag="wv", name="wv_all")
            nc.sync.dma_start(wv_all[:], wvT[:])
            pb_t = pp.tile([P, 12], F32, tag="pb", name="pb_t")
            nc.sync.dma_start(pb_t[:], pb[:])
            wo_all = pp.tile([P, NPT, DM], BF16, tag="wo", name="wo_all")
            nc.sync.dma_start(wo_all[:], woT[:])
            wk_t = [wk_all[:, i, :] for i in range(NXT)]
            wq_t = [wq_all[:, i, :] for i in range(NXT)]
            wv_t = [wv_all[:, i, :] for i in range(NXT)]
            xt = [xt_all[:, i, :] for i in range(NXT)]
            wo_t = [wo_all[:, j, :] for j in range(NPT)]

            # ---- persistent SBUF tensors ----
            KT = [pp.tile([P, S], BF16, tag=f"KT{j}", name=f"KT{j}") for j in range(NPT)]
            QT = [pp.tile([P, S], BF16, tag=f"QT{j}", name=f"QT{j}") for j in range(NPT)]
            V = [pp.tile([P, VROW], BF16, tag=f"V{c}", name=f"V{c}") for c in range(NKT)]
            attn = [pp.tile([P, S], BF16, tag=f"attn{j}", name=f"attn{j}") for j in range(NPT)]

            def proj(dst_tiles, w_tiles, bias_base, hp, qc):
                """dst[hp][:, qc] = w^T @ x + bias  (one [128,512] psum)."""
                ps = psp.tile([P, QC], F32, tag="acc", bufs=4, name=f"pj{bias_base}{hp}_{qc}")
                qsl = slice(qc * QC, (qc + 1) * QC)
                for kt in range(NXT):
                    nc.tensor.matmul(
                        ps[:],
                        w_tiles[kt][:, hp * P : (hp + 1) * P],
                        xt[kt][:, qsl],
                        start=(kt == 0),
                        stop=(kt == NXT - 1),
                    )
                nc.vector.tensor_scalar_add(
                    dst_tiles[hp][:, qsl], ps[:], pb_t[:, bias_base + hp : bias_base + hp + 1]
                )

            def vproj(c):
                """V[c] = (x_chunk @ Wv^T | ones) as [128, 6*65] bf16."""
                ps = psp.tile([P, QC], F32, tag="acc", bufs=4, name=f"vp{c}")
                for kt in range(NXT):
                    nc.tensor.matmul(
                        ps[:, 0:GD],
                        xt[kt][:, c * P : (c + 1) * P],
                        wv_t[kt][:],
                        start=(kt == 0),
                        stop=(kt == NXT - 1),
                    )
                vv = V[c].rearrange("p (h c) -> p h c", h=HLOC)
                nc.vector.tensor_copy(
                    vv[:, :, 0:DK], ps[:, 0:GD].rearrange("p (h c) -> p h c", h=HLOC)
                )
                nc.vector.memset(vv[:, :, DK : DK + 1], 1.0)

            def oproj(qc, mts):
                qsl = slice(qc * QC, (qc + 1) * QC)
                for mt in mts:
                    po = psp.tile([P, QC], F32, tag="acc", bufs=4, name=f"po{mt}_{qc}")
                    for j in range(NPT):
                        nc.tensor.matmul(
                            po[:],
                            wo_t[j][:, mt * P : (mt + 1) * P],
                            attn[j][:, qsl],
                            start=(j == 0),
                            stop=(j == NPT - 1),
                        )
                    osb = wp.tile([P, QC], BF16, tag="os", bufs=4, name=f"os{mt}_{qc}")
                    nc.vector.tensor_scalar_add(osb[:], po[:], pb_t[:, 6 + mt : 7 + mt])
                    nc.sync.dma_start(outT[mt * P : (mt + 1) * P, qsl], osb[:])

            # ---- emission helpers ----
            def oproj_mt(qc, mt):
                qsl = slice(qc * QC, (qc + 1) * QC)
                po = psp.tile([P, QC], F32, tag="acc", bufs=4, name=f"po{mt}_{qc}")
                for j in range(NPT):
                    nc.tensor.matmul(
                        po[:],
                        wo_t[j][:, mt * P : (mt + 1) * P],
                        attn[j][:, qsl],
                        start=(j == 0),
                        stop=(j == NPT - 1),
                    )
                osb = wp.tile([P, QC], BF16, tag="os", bufs=4, name=f"os{mt}_{qc}")
                nc.vector.tensor_scalar_add(osb[:], po[:], pb_t[:, 6 + mt : 7 + mt])
                nc.sync.dma_start(outT[mt * P : (mt + 1) * P, qsl], osb[:])

            def emit_pv(pv, c):
                E, atA, atB, hA, hB, _qc, _hp = pv
                nc.tensor.matmul(
                    atA[:], V[c][:, hA * VW : (hA + 1) * VW], E[c][:, 0:QC],
                    start=(c == 0), stop=(c == NKT - 1), skip_group_check=True,
                )
                nc.tensor.matmul(
                    atB[:], V[c][:, hB * VW : (hB + 1) * VW], E[c][:, QC : 2 * QC],
                    start=(c == 0), stop=(c == NKT - 1), skip_group_check=True,
                )

            def emit_normalize(pv):
                E, atA, atB, hA, hB, _qc, _hp = pv
                qsl = slice(_qc * QC, (_qc + 1) * QC)
                for at, rows in ((atA, slice(0, DK)), (atB, slice(DK, P))):
                    dsb = wp.tile([1, QC], F32, tag="dsb", bufs=4, name=f"d{_qc}_{_hp}_{rows.start}")
                    nc.vector.tensor_copy(dsb[:], at[DK:VW, :])
                    r = wp.tile([1, QC], F32, tag="r", bufs=4, name=f"r{_qc}_{_hp}_{rows.start}")
                    nc.vector.reciprocal_approx_fast(r[:], dsb[:])
                    rb = wp.tile([DK, QC], F32, tag="rb", bufs=4, name=f"rb{_qc}_{_hp}_{rows.start}")
                    nc.gpsimd.partition_broadcast(rb[:], r[:], channels=DK)
                    nc.vector.tensor_mul(attn[_hp][rows, qsl], at[0:DK, :], rb[:])

            # ---- prefix: only what the first score chunks need ----
            proj(KT, wk_t, 3, 0, 0)
            proj(QT, wq_t, 0, 0, 0)

            # ---- drip schedule: deferred PE work, one block per chunk slot ----
            drip = [[] for _ in range(12)]
            drip[0] = (
                [lambda q2=q2: proj(KT, wk_t, 3, 0, q2) for q2 in range(1, NQC)]
                + [lambda q2=q2: proj(KT, wk_t, 3, 1, q2) for q2 in range(NQC)]
                + [lambda: proj(QT, wq_t, 0, 1, 0)]
                + [lambda c=c: vproj(c) for c in range(8)]
            )
            drip[1] = (
                [lambda c=c: vproj(c) for c in range(8, NKT)]
                + [lambda q2=q2: proj(KT, wk_t, 3, 2, q2) for q2 in range(NQC)]
                + [lambda: proj(QT, wq_t, 0, 2, 0)]
            )
            drip[2] = [
                lambda: proj(QT, wq_t, 0, 0, 1),
                lambda: proj(QT, wq_t, 0, 1, 1),
                lambda: proj(QT, wq_t, 0, 2, 1),
            ]
            drip[3] = [lambda: proj(QT, wq_t, 0, 0, 2)]
            drip[4] = [lambda mt=mt: oproj_mt(0, mt) for mt in range(NXT)] + [
                lambda: proj(QT, wq_t, 0, 1, 2)
            ]
            drip[5] = [lambda: proj(QT, wq_t, 0, 2, 2), lambda: proj(QT, wq_t, 0, 0, 3)]
            drip[6] = [lambda: proj(QT, wq_t, 0, 1, 3)]
            drip[7] = [lambda mt=mt: oproj_mt(1, mt) for mt in range(NXT)] + [
                lambda: proj(QT, wq_t, 0, 2, 3)
            ]
            drip[10] = [lambda mt=mt: oproj_mt(2, mt) for mt in range(NXT)]

            # ---- attention: slot-pipelined emission ----
            prev = None
            iters = [(qc, hp) for qc in range(NQC) for hp in range(NPT)]
            for it_idx, (qc, hp) in enumerate(iters):
                qsl = slice(qc * QC, (qc + 1) * QC)
                hA, hB = 2 * hp, 2 * hp + 1
                E = []
                atA = psp.tile([VW, QC], F32, tag="acc", bufs=4, name=f"atA{qc}_{hp}")
                atB = psp.tile([VW, QC], F32, tag="acc", bufs=4, name=f"atB{qc}_{hp}")
                dq = list(drip[it_idx])
                for c in range(NKT):
                    sAB = psp.tile([P, 2 * QC], F32, tag="sAB", bufs=2, name=f"s{qc}_{hp}_{c}")
                    ksl = slice(c * P, (c + 1) * P)
                    nc.tensor.matmul(sAB[:, 0:QC], KT[hp][0:DK, ksl], QT[hp][0:DK, qsl])
                    nc.tensor.matmul(
                        sAB[:, QC : 2 * QC], KT[hp][DK:P, ksl], QT[hp][DK:P, qsl]
                    )
                    e = ep.tile([P, 2 * QC], BF16, tag="E", bufs=20, name=f"e{qc}_{hp}_{c}")
                    nc.scalar.activation(e[:], sAB[:], EXP, scale=1.0 / DK)
                    E.append(e)
                    if prev is not None:
                        emit_pv(prev, c)
                    if it_idx == len(iters) - 1 and c >= 1:
                        emit_pv((E, atA, atB, hA, hB, qc, hp), c - 1)
                    if dq:
                        dq.pop(0)()
                while dq:
                    dq.pop(0)()
                if prev is not None:
                    emit_normalize(prev)
                prev = (E, atA, atB, hA, hB, qc, hp)
            # flush last head-pair (chunks 0..14 already emitted in-loop)
            emit_pv(prev, NKT - 1)
            emit_normalize(prev)
            for mt in range(NXT):
                oproj_mt(3, mt)

    nc.compile()
    return nc


def make_in_maps(x, Wq, bq, Wk, bk, Wv, bv, Wo, bo):
    in_maps = []
    for c in range(NCORES):
        b, g = c // 2, c % 2
        sl = slice(g * GD, (g + 1) * GD)
        pbv = np.zeros((P, 12), np.float32)
        bo_eff = Wo[:, sl].astype(np.float64) @ bv[sl].astype(np.float64) + bo / 2.0
        for j in range(NPT):
            pbv[:, 0 + j] = bq[sl][j * P : (j + 1) * P]
            pbv[:, 3 + j] = bk[sl][j * P : (j + 1) * P]
        for j in range(NXT):
            pbv[:, 6 + j] = bo_eff[j * P : (j + 1) * P]
        def blk(a, rows):
            # [rows*P, C] -> [P, rows*C] with block i at cols [i*C:(i+1)*C]
            r, cdim = a.shape
            return np.ascontiguousarray(
                a.reshape(rows, P, cdim).transpose(1, 0, 2).reshape(P, rows * cdim)
            )

        in_maps.append(
            {
                "xT": blk(x[b].T, NXT).astype(NPBF16),
                "wqT": blk(Wq[sl, :].T, NXT).astype(NPBF16),
                "wkT": blk(Wk[sl, :].T, NXT).astype(NPBF16),
                "wvT": blk(Wv[sl, :].T, NXT).astype(NPBF16),
                "woT": blk(Wo[:, sl].T, NPT).astype(NPBF16),
                "pb": pbv,
            }
        )
    return in_maps


def kernel(x, Wq, bq, Wk, bk, Wv, bv, Wo, bo, _trace=False):
    x = np.asarray(x, np.float32)
    args = [np.asarray(a, np.float32) for a in (Wq, bq, Wk, bk, Wv, bv, Wo, bo)]
    if "nc" not in _NC_CACHE:
        _NC_CACHE["nc"] = build_nc()
    nc = _NC_CACHE["nc"]
    in_maps = make_in_maps(x, *args)
    res = run_bass_kernel_spmd(nc, in_maps, core_ids=list(range(NCORES)), trace=_trace)
    _NC_CACHE["last_result"] = res
    out = np.empty((B, S, DM), np.float32)
    for b in range(B):
        out[b] = (
            res.results[2 * b]["outT"].astype(np.float32)
            + res.results[2 * b + 1]["outT"].astype(np.float32)
        ).T
    return out


# revision 12
# speedup vs baseline: 1.0273x; 1.0273x over previous
"""Trainium2 Bass kernel for nn_MultiHeadAttention (B=4, S=2048, D=768, H=12).

Sharding: 8 cores = 4 batches x 2 head-groups (6 heads each).
Per core (batch b, group g), all bf16:
  KT/QT = Wg @ x_b^T            [384, 2048]  (bias added on DVE)
  V     = x_b @ Wv_g^T          16 tiles [128, 6*65] (65th col = 1.0 -> denom)
  attention per (qcp in 2, head-pair hp in 3):
    scores burst: 16 chunks; per chunk two row-tiled co-issued K=64 matmuls
      (head A rows 0-63, head B rows 64-127) -> sAB [128, 2048] PSUM
    exp: one ACT instr [128, 2048] per chunk -> E bf16 SBUF
    PV burst: per chunk 2 full-mode M=65 matmuls (V_aug^T @ E half),
      accumulating att[65, 1024] per head; row 64 = softmax denominator
    normalize: recip_approx_fast(denom row) -> gpsimd partition_broadcast
      -> DVE mult -> attn bf16
  o-proj per qcp: outT[768, 1024] bf16 (bias bo_eff = Wo_g @ bv_g + bo/2
  folded host-side; V projection carries no bias)
Host sums the two bf16 partial outT per batch (f32) and transposes back.
"""

import sys

import numpy as np
import ml_dtypes

if "/opt/trn_rl_repo" not in sys.path:
    sys.path.insert(0, "/opt/trn_rl_repo")

import concourse.bass as bass
import concourse.bacc as bacc
import concourse.mybir as mybir
import concourse.tile as tile
from concourse.bass_utils import run_bass_kernel_spmd

B, S, DM, NH, DK = 4, 2048, 768, 12, 64
NCORES = 8
HLOC = 6            # heads per core
GD = HLOC * DK      # 384
P = 128
NXT = DM // P       # 6 contraction tiles over d_model
NPT = GD // P       # 3 head-pair tiles (2 heads each)
NKT = S // P        # 16 k chunks
QC = 512            # q chunk
NQC = S // QC       # 4
VW = 65             # V cols per head (64 + ones)
VROW = HLOC * VW    # 390

F32 = mybir.dt.float32
BF16 = mybir.dt.bfloat16
FP8 = mybir.dt.float8e4
DR = mybir.MatmulPerfMode.DoubleRow
QK_FP8 = True
NPFP8 = ml_dtypes.float8_e4m3
EXP = mybir.ActivationFunctionType.Exp
NPBF16 = ml_dtypes.bfloat16

_NC_CACHE = {}


def build_nc():
    nc = bacc.Bacc()

    xT = nc.declare_dram_parameter("xT", [P, NXT * S], BF16, isOutput=False)
    if QK_FP8:
        x8T = nc.declare_dram_parameter("x8T", [P, 6 * S], FP8, isOutput=False)
        wq8T = nc.declare_dram_parameter("wq8T", [P, 6 * GD], FP8, isOutput=False)
        wk8T = nc.declare_dram_parameter("wk8T", [P, 6 * GD], FP8, isOutput=False)
    wqT = nc.declare_dram_parameter("wqT", [P, NXT * GD], BF16, isOutput=False)
    wkT = nc.declare_dram_parameter("wkT", [P, NXT * GD], BF16, isOutput=False)
    wvT = nc.declare_dram_parameter("wvT", [P, NXT * GD], BF16, isOutput=False)
    woT = nc.declare_dram_parameter("woT", [P, NPT * DM], BF16, isOutput=False)
    pb = nc.declare_dram_parameter("pb", [P, 12], F32, isOutput=False)
    outT = nc.declare_dram_parameter("outT", [DM, S], BF16, isOutput=True)

    with tile.TileContext(nc) as tc:
        with (
            nc.allow_low_precision(reason="bf16 pipeline is intended"),
            tc.tile_pool(name="persist", bufs=1) as pp,
            tc.tile_pool(name="xpool", bufs=1) as xp,
            tc.tile_pool(name="epool", bufs=1) as ep,
            tc.tile_pool(name="work", bufs=1) as wp,
            tc.tile_pool(name="psum", bufs=1, space=bass.MemorySpace.PSUM) as psp,
        ):
            # ---- DMA loads (one per tensor; sync dispatch is serialized) ----
            if QK_FP8:
                wk8_all = pp.tile([P, 3, 2, GD], FP8, tag="wk8", name="wk8_all")
                nc.sync.dma_start(wk8_all[:], wk8T[:])
                x8_all = xp.tile([P, 3, 2, S], FP8, tag="x8", name="x8_all")
                for i in range(3):
                    nc.sync.dma_start(x8_all[:, i, :, :], x8T[:, i * 2 * S : (i + 1) * 2 * S])
                wq8_all = pp.tile([P, 3, 2, GD], FP8, tag="wq8", name="wq8_all")
                nc.sync.dma_start(wq8_all[:], wq8T[:])
            wk_all = pp.tile([P, NXT, GD], BF16, tag="wk", name="wk_all")
            if not QK_FP8:
                nc.sync.dma_start(wk_all[:], wkT[:])
            xt_all = xp.tile([P, NXT, S], BF16, tag="xt", name="xt_all")
            for i in range(NXT):
                nc.sync.dma_start(xt_all[:, i, :], xT[:, i * S : (i + 1) * S])
            wq_all = pp.tile([P, NXT, GD], BF16, tag="wq", name="wq_all")
            if not QK_FP8:
                nc.sync.dma_start(wq_all[:], wqT[:])
            wv_all = pp.tile([P, NXT, GD], BF16, tag="wv", name="wv_all")
            nc.sync.dma_start(wv_all[:], wvT[:])
            pb_t = pp.tile([P, 12], F32, tag="pb", name="pb_t")
            nc.sync.dma_start(pb_t[:], pb[:])
            wo_all = pp.tile([P, NPT, DM], BF16, tag="wo", name="wo_all")
            nc.sync.dma_start(wo_all[:], woT[:])
            wk_t = [wk_all[:, i, :] for i in range(NXT)]
            wq_t = [wq_all[:, i, :] for i in range(NXT)]
            wv_t = [wv_all[:, i, :] for i in range(NXT)]
            xt = [xt_all[:, i, :] for i in range(NXT)]
            wo_t = [wo_all[:, j, :] for j in range(NPT)]

            # ---- persistent SBUF tensors ----
            KT = [pp.tile([P, S], BF16, tag=f"KT{j}", name=f"KT{j}") for j in range(NPT)]
            QT = [pp.tile([P, S], BF16, tag=f"QT{j}", name=f"QT{j}") for j in range(NPT)]
            V = [pp.tile([P, VROW], BF16, tag=f"V{c}", name=f"V{c}") for c in range(NKT)]
            attn = [pp.tile([P, S], BF16, tag=f"attn{j}", name=f"attn{j}") for j in range(NPT)]

            def proj(dst_tiles, w_tiles, bias_base, hp, qc):
                """dst[hp][:, qc] = w^T @ x + bias  (one [128,512] psum)."""
                ps = psp.tile([P, QC], F32, tag="acc", bufs=4, name=f"pj{bias_base}{hp}_{qc}")
                qsl = slice(qc * QC, (qc + 1) * QC)
                if QK_FP8 and w_tiles in (wq8_all, wk8_all):
                    for i in range(3):
                        nc.tensor.matmul(
                            ps[:],
                            w_tiles[:, i, :, hp * P : (hp + 1) * P],
                            x8_all[:, i, :, qsl],
                            start=(i == 0),
                            stop=(i == 2),
                            perf_mode=DR,
                        )
                else:
                    for kt in range(NXT):
                        nc.tensor.matmul(
                            ps[:],
                            w_tiles[kt][:, hp * P : (hp + 1) * P],
                            xt[kt][:, qsl],
                            start=(kt == 0),
                            stop=(kt == NXT - 1),
                        )
                nc.vector.tensor_scalar_add(
                    dst_tiles[hp][:, qsl], ps[:], pb_t[:, bias_base + hp : bias_base + hp + 1]
                )

            def vproj(c):
                """V[c] = (x_chunk @ Wv^T | ones) as [128, 6*65] bf16."""
                ps = psp.tile([P, QC], F32, tag="acc", bufs=4, name=f"vp{c}")
                for kt in range(NXT):
                    nc.tensor.matmul(
                        ps[:, 0:GD],
                        xt[kt][:, c * P : (c + 1) * P],
                        wv_t[kt][:],
                        start=(kt == 0),
                        stop=(kt == NXT - 1),
                    )
                vv = V[c].rearrange("p (h c) -> p h c", h=HLOC)
                nc.vector.tensor_copy(
                    vv[:, :, 0:DK], ps[:, 0:GD].rearrange("p (h c) -> p h c", h=HLOC)
                )
                nc.vector.memset(vv[:, :, DK : DK + 1], 1.0)

            def oproj(qc, mts):
                qsl = slice(qc * QC, (qc + 1) * QC)
                for mt in mts:
                    po = psp.tile([P, QC], F32, tag="acc", bufs=4, name=f"po{mt}_{qc}")
                    for j in range(NPT):
                        nc.tensor.matmul(
                            po[:],
                            wo_t[j][:, mt * P : (mt + 1) * P],
                            attn[j][:, qsl],
                            start=(j == 0),
                            stop=(j == NPT - 1),
                        )
                    osb = wp.tile([P, QC], BF16, tag="os", bufs=4, name=f"os{mt}_{qc}")
                    nc.vector.tensor_scalar_add(osb[:], po[:], pb_t[:, 6 + mt : 7 + mt])
                    nc.sync.dma_start(outT[mt * P : (mt + 1) * P, qsl], osb[:])

            # ---- emission helpers ----
            def oproj_mt(qc, mt):
                qsl = slice(qc * QC, (qc + 1) * QC)
                po = psp.tile([P, QC], F32, tag="acc", bufs=4, name=f"po{mt}_{qc}")
                for j in range(NPT):
                    nc.tensor.matmul(
                        po[:],
                        wo_t[j][:, mt * P : (mt + 1) * P],
                        attn[j][:, qsl],
                        start=(j == 0),
                        stop=(j == NPT - 1),
                    )
                osb = wp.tile([P, QC], BF16, tag="os", bufs=4, name=f"os{mt}_{qc}")
                nc.vector.tensor_scalar_add(osb[:], po[:], pb_t[:, 6 + mt : 7 + mt])
                nc.sync.dma_start(outT[mt * P : (mt + 1) * P, qsl], osb[:])

            def emit_pv(pv, c):
                E, atA, atB, hA, hB, _qc, _hp = pv
                nc.tensor.matmul(
                    atA[:], V[c][:, hA * VW : (hA + 1) * VW], E[c][:, 0:QC],
                    start=(c == 0), stop=(c == NKT - 1), skip_group_check=True,
                )
                nc.tensor.matmul(
                    atB[:], V[c][:, hB * VW : (hB + 1) * VW], E[c][:, QC : 2 * QC],
                    start=(c == 0), stop=(c == NKT - 1), skip_group_check=True,
                )

            def emit_normalize(pv):
                E, atA, atB, hA, hB, _qc, _hp = pv
                qsl = slice(_qc * QC, (_qc + 1) * QC)
                for at, rows in ((atA, slice(0, DK)), (atB, slice(DK, P))):
                    dsb = wp.tile([1, QC], F32, tag="dsb", bufs=4, name=f"d{_qc}_{_hp}_{rows.start}")
                    nc.vector.tensor_copy(dsb[:], at[DK:VW, :])
                    r = wp.tile([1, QC], F32, tag="r", bufs=4, name=f"r{_qc}_{_hp}_{rows.start}")
                    nc.vector.reciprocal_approx_fast(r[:], dsb[:])
                    rb = wp.tile([DK, QC], F32, tag="rb", bufs=4, name=f"rb{_qc}_{_hp}_{rows.start}")
                    nc.gpsimd.partition_broadcast(rb[:], r[:], channels=DK)
                    nc.vector.tensor_mul(attn[_hp][rows, qsl], at[0:DK, :], rb[:])

            # ---- prefix: only what the first score chunks need ----
            proj(KT, wk8_all if QK_FP8 else wk_t, 3, 0, 0)
            proj(QT, wq8_all if QK_FP8 else wq_t, 0, 0, 0)

            # ---- drip schedule: deferred PE work, one block per chunk slot ----
            drip = [[] for _ in range(12)]
            drip[0] = (
                [lambda q2=q2: proj(KT, wk8_all if QK_FP8 else wk_t, 3, 0, q2) for q2 in range(1, NQC)]
                + [lambda q2=q2: proj(KT, wk8_all if QK_FP8 else wk_t, 3, 1, q2) for q2 in range(NQC)]
                + [lambda: proj(QT, wq8_all if QK_FP8 else wq_t, 0, 1, 0)]
                + [lambda c=c: vproj(c) for c in range(8)]
            )
            drip[1] = (
                [lambda c=c: vproj(c) for c in range(8, NKT)]
                + [lambda q2=q2: proj(KT, wk8_all if QK_FP8 else wk_t, 3, 2, q2) for q2 in range(NQC)]
                + [lambda: proj(QT, wq8_all if QK_FP8 else wq_t, 0, 2, 0)]
            )
            drip[2] = [
                lambda: proj(QT, wq8_all if QK_FP8 else wq_t, 0, 0, 1),
                lambda: proj(QT, wq8_all if QK_FP8 else wq_t, 0, 1, 1),
                lambda: proj(QT, wq8_all if QK_FP8 else wq_t, 0, 2, 1),
            ]
            drip[3] = [lambda: proj(QT, wq8_all if QK_FP8 else wq_t, 0, 0, 2)]
            drip[4] = [lambda mt=mt: oproj_mt(0, mt) for mt in range(NXT)] + [
                lambda: proj(QT, wq8_all if QK_FP8 else wq_t, 0, 1, 2)
            ]
            drip[5] = [lambda: proj(QT, wq8_all if QK_FP8 else wq_t, 0, 2, 2), lambda: proj(QT, wq8_all if QK_FP8 else wq_t, 0, 0, 3)]
            drip[6] = [lambda: proj(QT, wq8_all if QK_FP8 else wq_t, 0, 1, 3)]
            drip[7] = [lambda mt=mt: oproj_mt(1, mt) for mt in range(NXT)] + [
                lambda: proj(QT, wq8_all if QK_FP8 else wq_t, 0, 2, 3)
            ]
            drip[10] = [lambda mt=mt: oproj_mt(2, mt) for mt in range(NXT)]

            # ---- attention: slot-pipelined emission ----
            prev = None
            iters = [(qc, hp) for qc in range(NQC) for hp in range(NPT)]
            for it_idx, (qc, hp) in enumerate(iters):
                qsl = slice(qc * QC, (qc + 1) * QC)
                hA, hB = 2 * hp, 2 * hp + 1
                E = []
                atA = psp.tile([VW, QC], F32, tag="acc", bufs=4, name=f"atA{qc}_{hp}")
                atB = psp.tile([VW, QC], F32, tag="acc", bufs=4, name=f"atB{qc}_{hp}")
                dq = list(drip[it_idx])
                for c in range(NKT):
                    sAB = psp.tile([P, 2 * QC], F32, tag="sAB", bufs=2, name=f"s{qc}_{hp}_{c}")
                    ksl = slice(c * P, (c + 1) * P)
                    nc.tensor.matmul(sAB[:, 0:QC], KT[hp][0:DK, ksl], QT[hp][0:DK, qsl])
                    nc.tensor.matmul(
                        sAB[:, QC : 2 * QC], KT[hp][DK:P, ksl], QT[hp][DK:P, qsl]
                    )
                    e = ep.tile([P, 2 * QC], BF16, tag="E", bufs=20, name=f"e{qc}_{hp}_{c}")
                    nc.scalar.activation(e[:], sAB[:], EXP, scale=(1.0 / (DK * DK * 1.0)) if QK_FP8 else (1.0 / DK))
                    E.append(e)
                    if prev is not None:
                        emit_pv(prev, c)
                    if it_idx == len(iters) - 1 and c >= 1:
                        emit_pv((E, atA, atB, hA, hB, qc, hp), c - 1)
                    if dq:
                        dq.pop(0)()
                while dq:
                    dq.pop(0)()
                if prev is not None:
                    emit_normalize(prev)
                prev = (E, atA, atB, hA, hB, qc, hp)
            # flush last head-pair (chunks 0..14 already emitted in-loop)
            emit_pv(prev, NKT - 1)
            emit_normalize(prev)
            for mt in range(NXT):
                oproj_mt(3, mt)

    nc.compile()
    return nc


def make_in_maps(x, Wq, bq, Wk, bk, Wv, bv, Wo, bo):
    in_maps = []
    for c in range(NCORES):
        b, g = c // 2, c % 2
        sl = slice(g * GD, (g + 1) * GD)
        pbv = np.zeros((P, 12), np.float32)
        bo_eff = Wo[:, sl].astype(np.float64) @ bv[sl].astype(np.float64) + bo / 2.0
        qs = 8.0 if QK_FP8 else 1.0
        for j in range(NPT):
            pbv[:, 0 + j] = bq[sl][j * P : (j + 1) * P] * qs
            pbv[:, 3 + j] = bk[sl][j * P : (j + 1) * P] * qs
        for j in range(NXT):
            pbv[:, 6 + j] = bo_eff[j * P : (j + 1) * P]
        def blk(a, rows):
            # [rows*P, C] -> [P, rows*C] with block i at cols [i*C:(i+1)*C]
            r, cdim = a.shape
            return np.ascontiguousarray(
                a.reshape(rows, P, cdim).transpose(1, 0, 2).reshape(P, rows * cdim)
            )

        m = {}
        if QK_FP8:
            def dr_blk(a):
                # [768, C] -> [P, 3, 2, C] with d = i*256 + t*128 + p
                cdim = a.shape[1]
                return np.ascontiguousarray(
                    a.reshape(3, 2, P, cdim).transpose(2, 0, 1, 3).reshape(P, 3 * 2 * cdim)
                )
            m["x8T"] = dr_blk(x[b].T).astype(NPFP8)
            m["wq8T"] = dr_blk(Wq[sl, :].T * 8.0).astype(NPFP8)
            m["wk8T"] = dr_blk(Wk[sl, :].T * 8.0).astype(NPFP8)
        in_maps.append(
            {
                **m,
                "xT": blk(x[b].T, NXT).astype(NPBF16),
                "wqT": blk(Wq[sl, :].T, NXT).astype(NPBF16),
                "wkT": blk(Wk[sl, :].T, NXT).astype(NPBF16),
                "wvT": blk(Wv[sl, :].T, NXT).astype(NPBF16),
                "woT": blk(Wo[:, sl].T, NPT).astype(NPBF16),
                "pb": pbv,
            }
        )
    return in_maps


def kernel(x, Wq, bq, Wk, bk, Wv, bv, Wo, bo, _trace=False):
    x = np.asarray(x, np.float32)
    args = [np.asarray(a, np.float32) for a in (Wq, bq, Wk, bk, Wv, bv, Wo, bo)]
    if "nc" not in _NC_CACHE:
        _NC_CACHE["nc"] = build_nc()
    nc = _NC_CACHE["nc"]
    in_maps = make_in_maps(x, *args)
    res = run_bass_kernel_spmd(nc, in_maps, core_ids=list(range(NCORES)), trace=_trace)
    _NC_CACHE["last_result"] = res
    out = np.empty((B, S, DM), np.float32)
    for b in range(B):
        out[b] = (
            res.results[2 * b]["outT"].astype(np.float32)
            + res.results[2 * b + 1]["outT"].astype(np.float32)
        ).T
    return out


# revision 14
# speedup vs baseline: 1.0523x; 1.0243x over previous
"""Trainium2 Bass kernel for nn_MultiHeadAttention (B=4, S=2048, D=768, H=12).

Sharding: 8 cores = 4 batches x 2 head-groups (6 heads each).
Per core (batch b, group g), all bf16:
  KT/QT = Wg @ x_b^T            [384, 2048]  (bias added on DVE)
  V     = x_b @ Wv_g^T          16 tiles [128, 6*65] (65th col = 1.0 -> denom)
  attention per (qcp in 2, head-pair hp in 3):
    scores burst: 16 chunks; per chunk two row-tiled co-issued K=64 matmuls
      (head A rows 0-63, head B rows 64-127) -> sAB [128, 2048] PSUM
    exp: one ACT instr [128, 2048] per chunk -> E bf16 SBUF
    PV burst: per chunk 2 full-mode M=65 matmuls (V_aug^T @ E half),
      accumulating att[65, 1024] per head; row 64 = softmax denominator
    normalize: recip_approx_fast(denom row) -> gpsimd partition_broadcast
      -> DVE mult -> attn bf16
  o-proj per qcp: outT[768, 1024] bf16 (bias bo_eff = Wo_g @ bv_g + bo/2
  folded host-side; V projection carries no bias)
Host sums the two bf16 partial outT per batch (f32) and transposes back.
"""

import sys

import numpy as np
import ml_dtypes

if "/opt/trn_rl_repo" not in sys.path:
    sys.path.insert(0, "/opt/trn_rl_repo")

import concourse.bass as bass
import concourse.bacc as bacc
import concourse.mybir as mybir
import concourse.tile as tile
from concourse.bass_utils import run_bass_kernel_spmd

B, S, DM, NH, DK = 4, 2048, 768, 12, 64
NCORES = 8
HLOC = 6            # heads per core
GD = HLOC * DK      # 384
P = 128
NXT = DM // P       # 6 contraction tiles over d_model
NPT = GD // P       # 3 head-pair tiles (2 heads each)
NKT = S // P        # 16 k chunks
QC = 512            # q chunk
NQC = S // QC       # 4
VW = 65             # V cols per head (64 + ones)
VROW = HLOC * VW    # 390

F32 = mybir.dt.float32
BF16 = mybir.dt.bfloat16
FP8 = mybir.dt.float8e4
DR = mybir.MatmulPerfMode.DoubleRow
QK_FP8 = True
NPFP8 = ml_dtypes.float8_e4m3
EXP = mybir.ActivationFunctionType.Exp
NPBF16 = ml_dtypes.bfloat16

_NC_CACHE = {}


def build_nc():
    nc = bacc.Bacc()

    xT = nc.declare_dram_parameter("xT", [P, NXT * S], BF16, isOutput=False)
    if QK_FP8:
        x8T = nc.declare_dram_parameter("x8T", [P, 6 * S], FP8, isOutput=False)
        wq8T = nc.declare_dram_parameter("wq8T", [P, 6 * GD], FP8, isOutput=False)
        wk8T = nc.declare_dram_parameter("wk8T", [P, 6 * GD], FP8, isOutput=False)
    wqT = nc.declare_dram_parameter("wqT", [P, NXT * GD], BF16, isOutput=False)
    wkT = nc.declare_dram_parameter("wkT", [P, NXT * GD], BF16, isOutput=False)
    wvT = nc.declare_dram_parameter("wvT", [P, NXT * GD], BF16, isOutput=False)
    woT = nc.declare_dram_parameter("woT", [P, NPT * DM], BF16, isOutput=False)
    pb = nc.declare_dram_parameter("pb", [P, 12], F32, isOutput=False)
    outT = nc.declare_dram_parameter("outT", [DM, S], BF16, isOutput=True)

    with tile.TileContext(nc) as tc:
        with (
            nc.allow_low_precision(reason="bf16 pipeline is intended"),
            tc.tile_pool(name="persist", bufs=1) as pp,
            tc.tile_pool(name="xpool", bufs=1) as xp,
            tc.tile_pool(name="epool", bufs=1) as ep,
            tc.tile_pool(name="work", bufs=1) as wp,
            tc.tile_pool(name="psum", bufs=1, space=bass.MemorySpace.PSUM) as psp,
        ):
            # ---- DMA loads (one per tensor; sync dispatch is serialized) ----
            pb_t = pp.tile([P, 12], F32, tag="pb", name="pb_t")
            nc.sync.dma_start(pb_t[:], pb[:])
            if QK_FP8:
                wk8_all = pp.tile([P, 3, 2, GD], FP8, tag="wk8", name="wk8_all")
                nc.sync.dma_start(wk8_all[:], wk8T[:])
                x8_all = xp.tile([P, 3, 2, S], FP8, tag="x8", name="x8_all")
                for i in range(3):
                    nc.sync.dma_start(x8_all[:, i, :, :], x8T[:, i * 2 * S : (i + 1) * 2 * S])
                wq8_all = pp.tile([P, 3, 2, GD], FP8, tag="wq8", name="wq8_all")
                nc.sync.dma_start(wq8_all[:], wq8T[:])
            wk_all = pp.tile([P, NXT, GD], BF16, tag="wk", name="wk_all")
            if not QK_FP8:
                nc.sync.dma_start(wk_all[:], wkT[:])
            xt_all = xp.tile([P, NXT, S], BF16, tag="xt", name="xt_all")
            for i in range(NXT):
                nc.sync.dma_start(xt_all[:, i, :], xT[:, i * S : (i + 1) * S])
            wq_all = pp.tile([P, NXT, GD], BF16, tag="wq", name="wq_all")
            if not QK_FP8:
                nc.sync.dma_start(wq_all[:], wqT[:])
            wv_all = pp.tile([P, NXT, GD], BF16, tag="wv", name="wv_all")
            nc.sync.dma_start(wv_all[:], wvT[:])
            wo_all = pp.tile([P, NPT, DM], BF16, tag="wo", name="wo_all")
            nc.sync.dma_start(wo_all[:], woT[:])
            wk_t = [wk_all[:, i, :] for i in range(NXT)]
            wq_t = [wq_all[:, i, :] for i in range(NXT)]
            wv_t = [wv_all[:, i, :] for i in range(NXT)]
            xt = [xt_all[:, i, :] for i in range(NXT)]
            wo_t = [wo_all[:, j, :] for j in range(NPT)]

            # ---- persistent SBUF tensors ----
            KT = [pp.tile([P, S], BF16, tag=f"KT{j}", name=f"KT{j}") for j in range(NPT)]
            QT = [pp.tile([P, S], BF16, tag=f"QT{j}", name=f"QT{j}") for j in range(NPT)]
            V = [pp.tile([P, VROW], BF16, tag=f"V{c}", name=f"V{c}") for c in range(NKT)]
            attn = [pp.tile([P, S], BF16, tag=f"attn{j}", name=f"attn{j}") for j in range(NPT)]

            def proj(dst_tiles, w_tiles, bias_base, hp, qc):
                """dst[hp][:, qc] = w^T @ x + bias  (one [128,512] psum)."""
                ps = psp.tile([P, QC], F32, tag="acc", bufs=4, name=f"pj{bias_base}{hp}_{qc}")
                qsl = slice(qc * QC, (qc + 1) * QC)
                if QK_FP8 and w_tiles in (wq8_all, wk8_all):
                    for i in range(3):
                        nc.tensor.matmul(
                            ps[:],
                            w_tiles[:, i, :, hp * P : (hp + 1) * P],
                            x8_all[:, i, :, qsl],
                            start=(i == 0),
                            stop=(i == 2),
                            perf_mode=DR,
                        )
                else:
                    for kt in range(NXT):
                        nc.tensor.matmul(
                            ps[:],
                            w_tiles[kt][:, hp * P : (hp + 1) * P],
                            xt[kt][:, qsl],
                            start=(kt == 0),
                            stop=(kt == NXT - 1),
                        )
                nc.vector.tensor_scalar_add(
                    dst_tiles[hp][:, qsl], ps[:], pb_t[:, bias_base + hp : bias_base + hp + 1]
                )

            def vproj(c):
                """V[c] = (x_chunk @ Wv^T | ones) as [128, 6*65] bf16."""
                ps = psp.tile([P, QC], F32, tag="acc", bufs=4, name=f"vp{c}")
                for kt in range(NXT):
                    nc.tensor.matmul(
                        ps[:, 0:GD],
                        xt[kt][:, c * P : (c + 1) * P],
                        wv_t[kt][:],
                        start=(kt == 0),
                        stop=(kt == NXT - 1),
                    )
                vv = V[c].rearrange("p (h c) -> p h c", h=HLOC)
                nc.vector.tensor_copy(
                    vv[:, :, 0:DK], ps[:, 0:GD].rearrange("p (h c) -> p h c", h=HLOC)
                )
                nc.vector.memset(vv[:, :, DK : DK + 1], 1.0)

            def oproj(qc, mts):
                qsl = slice(qc * QC, (qc + 1) * QC)
                for mt in mts:
                    po = psp.tile([P, QC], F32, tag="acc", bufs=4, name=f"po{mt}_{qc}")
                    for j in range(NPT):
                        nc.tensor.matmul(
                            po[:],
                            wo_t[j][:, mt * P : (mt + 1) * P],
                            attn[j][:, qsl],
                            start=(j == 0),
                            stop=(j == NPT - 1),
                        )
                    osb = wp.tile([P, QC], BF16, tag="os", bufs=4, name=f"os{mt}_{qc}")
                    nc.vector.tensor_scalar_add(osb[:], po[:], pb_t[:, 6 + mt : 7 + mt])
                    nc.sync.dma_start(outT[mt * P : (mt + 1) * P, qsl], osb[:])

            # ---- emission helpers ----
            def oproj_mt(qc, mt):
                qsl = slice(qc * QC, (qc + 1) * QC)
                po = psp.tile([P, QC], F32, tag="acc", bufs=4, name=f"po{mt}_{qc}")
                for j in range(NPT):
                    nc.tensor.matmul(
                        po[:],
                        wo_t[j][:, mt * P : (mt + 1) * P],
                        attn[j][:, qsl],
                        start=(j == 0),
                        stop=(j == NPT - 1),
                    )
                osb = wp.tile([P, QC], BF16, tag="os", bufs=4, name=f"os{mt}_{qc}")
                nc.vector.tensor_scalar_add(osb[:], po[:], pb_t[:, 6 + mt : 7 + mt])
                nc.sync.dma_start(outT[mt * P : (mt + 1) * P, qsl], osb[:])

            def emit_pv(pv, c):
                E, atA, atB, hA, hB, _qc, _hp = pv
                nc.tensor.matmul(
                    atA[:], V[c][:, hA * VW : (hA + 1) * VW], E[c][:, 0:QC],
                    start=(c == 0), stop=(c == NKT - 1), skip_group_check=True,
                )
                nc.tensor.matmul(
                    atB[:], V[c][:, hB * VW : (hB + 1) * VW], E[c][:, QC : 2 * QC],
                    start=(c == 0), stop=(c == NKT - 1), skip_group_check=True,
                )

            def emit_normalize(pv):
                E, atA, atB, hA, hB, _qc, _hp = pv
                qsl = slice(_qc * QC, (_qc + 1) * QC)
                for at, rows in ((atA, slice(0, DK)), (atB, slice(DK, P))):
                    dsb = wp.tile([1, QC], F32, tag="dsb", bufs=4, name=f"d{_qc}_{_hp}_{rows.start}")
                    nc.vector.tensor_copy(dsb[:], at[DK:VW, :])
                    r = wp.tile([1, QC], F32, tag="r", bufs=4, name=f"r{_qc}_{_hp}_{rows.start}")
                    nc.vector.reciprocal_approx_fast(r[:], dsb[:])
                    rb = wp.tile([DK, QC], F32, tag="rb", bufs=4, name=f"rb{_qc}_{_hp}_{rows.start}")
                    nc.gpsimd.partition_broadcast(rb[:], r[:], channels=DK)
                    nc.vector.tensor_mul(attn[_hp][rows, qsl], at[0:DK, :], rb[:])

            # ---- prefix: only what the first score chunks need ----
            proj(KT, wk8_all if QK_FP8 else wk_t, 3, 0, 0)
            proj(QT, wq8_all if QK_FP8 else wq_t, 0, 0, 0)

            # ---- drip schedule: deferred PE work, one block per chunk slot ----
            drip = [[] for _ in range(12)]
            drip[0] = (
                [lambda q2=q2: proj(KT, wk8_all if QK_FP8 else wk_t, 3, 0, q2) for q2 in range(1, NQC)]
                + [lambda q2=q2: proj(KT, wk8_all if QK_FP8 else wk_t, 3, 1, q2) for q2 in range(NQC)]
                + [lambda: proj(QT, wq8_all if QK_FP8 else wq_t, 0, 1, 0)]
                + [lambda c=c: vproj(c) for c in range(8)]
            )
            drip[1] = (
                [lambda c=c: vproj(c) for c in range(8, 12)]
                + [lambda: proj(KT, wk8_all if QK_FP8 else wk_t, 3, 2, 0), lambda: proj(QT, wq8_all if QK_FP8 else wq_t, 0, 2, 0)]
                + [lambda c=c: vproj(c) for c in range(12, NKT)]
            )
            drip[2] = (
                [lambda q2=q2: proj(KT, wk8_all if QK_FP8 else wk_t, 3, 2, q2) for q2 in range(1, NQC)]
                + [lambda: proj(QT, wq8_all if QK_FP8 else wq_t, 0, 0, 1)]
            )
            drip[3] = [lambda: proj(QT, wq8_all if QK_FP8 else wq_t, 0, 1, 1), lambda: proj(QT, wq8_all if QK_FP8 else wq_t, 0, 2, 1)]
            drip[4] = [lambda mt=mt: oproj_mt(0, mt) for mt in range(NXT)] + [
                lambda: proj(QT, wq8_all if QK_FP8 else wq_t, 0, 0, 2)
            ]
            drip[5] = [lambda: proj(QT, wq8_all if QK_FP8 else wq_t, 0, 1, 2), lambda: proj(QT, wq8_all if QK_FP8 else wq_t, 0, 2, 2)]
            drip[6] = [lambda: proj(QT, wq8_all if QK_FP8 else wq_t, 0, 0, 3)]
            drip[7] = [lambda mt=mt: oproj_mt(1, mt) for mt in range(NXT)] + [
                lambda: proj(QT, wq8_all if QK_FP8 else wq_t, 0, 1, 3)
            ]
            drip[8] = [lambda: proj(QT, wq8_all if QK_FP8 else wq_t, 0, 2, 3)]
            drip[10] = [lambda mt=mt: oproj_mt(2, mt) for mt in range(NXT)]

            # ---- attention: slot-pipelined emission ----
            prev = None
            iters = [(qc, hp) for qc in range(NQC) for hp in range(NPT)]
            for it_idx, (qc, hp) in enumerate(iters):
                qsl = slice(qc * QC, (qc + 1) * QC)
                hA, hB = 2 * hp, 2 * hp + 1
                E = []
                atA = psp.tile([VW, QC], F32, tag="acc", bufs=4, name=f"atA{qc}_{hp}")
                atB = psp.tile([VW, QC], F32, tag="acc", bufs=4, name=f"atB{qc}_{hp}")
                dq = list(drip[it_idx])
                for c in range(NKT):
                    sAB = psp.tile([P, 2 * QC], F32, tag="sAB", bufs=2, name=f"s{qc}_{hp}_{c}")
                    ksl = slice(c * P, (c + 1) * P)
                    nc.tensor.matmul(sAB[:, 0:QC], KT[hp][0:DK, ksl], QT[hp][0:DK, qsl])
                    nc.tensor.matmul(
                        sAB[:, QC : 2 * QC], KT[hp][DK:P, ksl], QT[hp][DK:P, qsl]
                    )
                    e = ep.tile([P, 2 * QC], BF16, tag="E", bufs=20, name=f"e{qc}_{hp}_{c}")
                    nc.scalar.activation(e[:], sAB[:], EXP, scale=(1.0 / (DK * DK * 1.0)) if QK_FP8 else (1.0 / DK))
                    E.append(e)
                    if prev is not None:
                        emit_pv(prev, c)
                    if it_idx == len(iters) - 1 and c >= 1:
                        emit_pv((E, atA, atB, hA, hB, qc, hp), c - 1)
                    if dq:
                        dq.pop(0)()
                while dq:
                    dq.pop(0)()
                if prev is not None:
                    emit_normalize(prev)
                prev = (E, atA, atB, hA, hB, qc, hp)
            # flush last head-pair (chunks 0..14 already emitted in-loop)
            emit_pv(prev, NKT - 1)
            emit_normalize(prev)
            for mt in range(NXT):
                oproj_mt(3, mt)

    nc.compile()
    return nc


def make_in_maps(x, Wq, bq, Wk, bk, Wv, bv, Wo, bo):
    in_maps = []
    for c in range(NCORES):
        b, g = c // 2, c % 2
        sl = slice(g * GD, (g + 1) * GD)
        pbv = np.zeros((P, 12), np.float32)
        bo_eff = Wo[:, sl].astype(np.float64) @ bv[sl].astype(np.float64) + bo / 2.0
        qs = 8.0 if QK_FP8 else 1.0
        for j in range(NPT):
            pbv[:, 0 + j] = bq[sl][j * P : (j + 1) * P] * qs
            pbv[:, 3 + j] = bk[sl][j * P : (j + 1) * P] * qs
        for j in range(NXT):
            pbv[:, 6 + j] = bo_eff[j * P : (j + 1) * P]
        def blk(a, rows):
            # [rows*P, C] -> [P, rows*C] with block i at cols [i*C:(i+1)*C]
            r, cdim = a.shape
            return np.ascontiguousarray(
                a.reshape(rows, P, cdim).transpose(1, 0, 2).reshape(P, rows * cdim)
            )

        m = {}
        if QK_FP8:
            def dr_blk(a):
                # [768, C] -> [P, 3, 2, C] with d = i*256 + t*128 + p
                cdim = a.shape[1]
                return np.ascontiguousarray(
                    a.reshape(3, 2, P, cdim).transpose(2, 0, 1, 3).reshape(P, 3 * 2 * cdim)
                )
            m["x8T"] = dr_blk(x[b].T).astype(NPFP8)
            m["wq8T"] = dr_blk(Wq[sl, :].T * 8.0).astype(NPFP8)
            m["wk8T"] = dr_blk(Wk[sl, :].T * 8.0).astype(NPFP8)
        in_maps.append(
            {
                **m,
                "xT": blk(x[b].T, NXT).astype(NPBF16),
                "wqT": blk(Wq[sl, :].T, NXT).astype(NPBF16),
                "wkT": blk(Wk[sl, :].T, NXT).astype(NPBF16),
                "wvT": blk(Wv[sl, :].T, NXT).astype(NPBF16),
                "woT": blk(Wo[:, sl].T, NPT).astype(NPBF16),
                "pb": pbv,
            }
        )
    return in_maps


def kernel(x, Wq, bq, Wk, bk, Wv, bv, Wo, bo, _trace=False):
    x = np.asarray(x, np.float32)
    args = [np.asarray(a, np.float32) for a in (Wq, bq, Wk, bk, Wv, bv, Wo, bo)]
    if "nc" not in _NC_CACHE:
        _NC_CACHE["nc"] = build_nc()
    nc = _NC_CACHE["nc"]
    in_maps = make_in_maps(x, *args)
    res = run_bass_kernel_spmd(nc, in_maps, core_ids=list(range(NCORES)), trace=_trace)
    _NC_CACHE["last_result"] = res
    out = np.empty((B, S, DM), np.float32)
    for b in range(B):
        out[b] = (
            res.results[2 * b]["outT"].astype(np.float32)
            + res.results[2 * b + 1]["outT"].astype(np.float32)
        ).T
    return out
